# revision 1
# baseline (speedup 1.0000x reference)
"""DGI (2-layer GCN encoder + bilinear disc) Bass kernel for trn2, 8-core SPMD.

Pipeline per core (shard = W windows of 128 dst nodes):
  pass0: y1 = x_sh @ W1, z1 = dinv*y1  (and xp -> z3)  -> z13_sh bf16 [SH, 256]
  AG1:   z13_full [NP, 256]
  pass1: conv via gather+S-matmul from z13_full -> h1p, h1n -> z24_sh bf16
  AG2:   z24_full
  pass2: conv from z24_full -> H, Hc (f32, DRAM) + masked readout partial
  AR:    readout sum [128] -> s = sigmoid(sum/N) -> Ws = Wd0@s
  final: pos = H@Ws + bd, neg = Hc@Ws + bd
"""
import numpy as np
import ml_dtypes

import concourse.bacc as bacc
import concourse.mybir as mybir
import concourse.tile as tile
from concourse.bass_utils import run_bass_kernel_spmd
from concourse.library_config import mlp as mlp_lib

P = 128
F = 128          # hidden/out features
FIN = 512        # input features
C = 8            # cores
GW = 4           # windows per PSUM group
MAXG = 4096      # max idxs per dma_gather

BF16 = mybir.dt.bfloat16
F32 = mybir.dt.float32
I16 = mybir.dt.int16


# ---------------------------------------------------------------- host plan --
def build_plan(src, dst, n_real, w_per_core, bucket_rows):
    """Common (cross-core) chunk plan + per-core idx/dstv arrays.

    src, dst: int32 arrays (self-loops already appended).
    Returns dict with plan structure and per-core data.
    """
    SH = w_per_core * P
    NP = SH * C
    W = w_per_core
    NB = (NP + bucket_rows - 1) // bucket_rows
    core = dst // SH
    wloc = (dst % SH) // P
    buck = src // bucket_rows

    # counts[c, w, b]
    counts = np.zeros((C, W, NB), np.int64)
    np.add.at(counts, (core, wloc, buck), 1)
    K = np.maximum(np.ceil(counts.max(axis=0) / P).astype(np.int64), 0)  # [W, NB]
    # ensure every window has >= 1 chunk so start/stop flags exist
    for w in range(W):
        if K[w].sum() == 0:
            K[w][0] = 1

    NCH = int(K.sum())
    S = NCH * P

    # slot order: for g (groups of GW windows): for b: for w in g: K[w,b] chunks
    # chunk_list entries: (w, b); gather_list: (slot_off, n_idx, b)
    chunk_list = []
    gather_list = []
    ngroups = (W + GW - 1) // GW
    for g in range(ngroups):
        ws = range(g * GW, min((g + 1) * GW, W))
        for b in range(NB):
            run_chunks = sum(int(K[w, b]) for w in ws)
            if run_chunks == 0:
                continue
            run_off = len(chunk_list) * P
            for w in ws:
                chunk_list.extend([(w, b)] * int(K[w, b]))
            # split run into gathers
            left = run_chunks * P
            off = run_off
            while left > 0:
                n = min(left, MAXG)
                gather_list.append((off, n, b))
                off += n
                left -= n

    # per-window first/last chunk index (for start/stop flags)
    first = {}
    last = {}
    for ci, (w, b) in enumerate(chunk_list):
        if w not in first:
            first[w] = ci
        last[w] = ci

    # per-core arrays
    order = np.lexsort((dst, buck, wloc, core))
    so, do = src[order], dst[order]
    co, wo, bo = core[order], wloc[order], buck[order]
    idx_all = np.zeros((C, S), np.int16)
    dstv_all = np.full((C, S), -1e6, np.float32)
    # slot offsets per (w, b) in common layout
    slot_off = {}
    cur = 0
    for ci, (w, b) in enumerate(chunk_list):
        if (w, b) not in slot_off:
            slot_off[(w, b)] = cur
        cur += P
    for c in range(C):
        m = co == c
        s_c, d_c, w_c, b_c = so[m], do[m], wo[m], bo[m]
        # group edges by (w, b) — they are contiguous after lexsort per core
        keys = w_c.astype(np.int64) * NB + b_c
        uniq, starts = np.unique(keys, return_index=True)
        ends = np.append(starts[1:], len(keys))
        for u, st, en in zip(uniq, starts, ends):
            w, b = int(u) // NB, int(u) % NB
            n = en - st
            off = slot_off[(w, b)]
            idx_all[c, off:off + n] = (s_c[st:en] - b * bucket_rows).astype(np.int16)
            dstv_all[c, off:off + n] = (d_c[st:en] - c * SH - w * P).astype(np.float32)

    # wrapped idx layout [128, S/16]: within each gather block of n idxs,
    # idx i lives at [i%16, (block_col_off) + i//16]
    idx_wr = np.zeros((C, 128, S // 16), np.int16)
    for (off, n, b) in gather_list:
        col0 = off // 16
        for c in range(C):
            blk = idx_all[c, off:off + n].reshape(n // 16, 16).T  # [16, n/16]
            idx_wr[c, :, col0:col0 + n // 16] = np.tile(blk, (8, 1))
    dstv = dstv_all.reshape(C, NCH, P).transpose(0, 2, 1).copy()  # [C, 128, NCH]

    return dict(SH=SH, NP=NP, W=W, NB=NB, NCH=NCH, S=S,
                bucket_rows=bucket_rows, K=K, chunk_list=chunk_list,
                gather_list=gather_list, first=first, last=last,
                idx_wr=idx_wr, dstv=dstv, ngroups=ngroups)


# ------------------------------------------------------------- bass builder --
def conv_pass(nc, tc, plan, z_full, dinv_sb, iota_f, dstv_sb, idx_dram,
              pools, evict_fn):
    """Emit one conv pass: gathers + S-matmul chunk loop + per-window evict.

    evict_fn(w, accA_ap, accB_ap): consume PSUM slices for window w.
    """
    gath_pool, s_pool, idxt_pool, acc_pool = pools
    W, NB, GW_ = plan["W"], plan["NB"], GW
    chunk_list, gather_list = plan["chunk_list"], plan["gather_list"]
    first, last = plan["first"], plan["last"]
    BR = plan["bucket_rows"]
    NPr = plan["NP"]

    # iterate chunks; issue gathers when crossing gather boundaries
    gi = 0            # next gather to issue
    cur_tiles = []    # list of (start_chunk, nchunks, tile)
    acc_tiles = {}    # w -> (tile, col_off)

    def ensure_gather(ci):
        nonlocal gi
        while gi < len(gather_list) and gather_list[gi][0] <= ci * P:
            off, n, b = gather_list[gi]
            gt = gath_pool.tile([P, MAXG // P, 2 * F], BF16, tag="gath")
            it = idxt_pool.tile([P, MAXG // 16], I16, tag="idxt")
            nc.sync.dma_start(it[:, :n // 16], idx_dram[:, off // 16:(off + n) // 16])
            lo = b * BR
            hi = min(lo + BR, NPr)
            nc.gpsimd.dma_gather(
                gt[:, :n // P, :], z_full[lo:hi, :], it[:, :n // 16],
                num_idxs=n, num_idxs_reg=n, elem_size=2 * F,
                single_packet=False)
            cur_tiles.append((off // P, n // P, gt))
            if len(cur_tiles) > 3:
                cur_tiles.pop(0)
            gi += 1

    for ci, (w, b) in enumerate(chunk_list):
        ensure_gather(ci)
        # find tile holding chunk ci
        gt = None
        for (c0, nch, t) in cur_tiles:
            if c0 <= ci < c0 + nch:
                gt = t
                slot = ci - c0
                break
        assert gt is not None
        if w not in acc_tiles:
            acc_tiles[w] = acc_pool.tile([P, 2 * F], F32, tag="acc",
                                         name=f"acc_w{w}")
        bank = acc_tiles[w]
        s_t = s_pool.tile([P, P], BF16, tag="s")
        nc.vector.tensor_tensor(
            out=s_t[:],
            in0=dstv_sb[:, ci:ci + 1].to_broadcast((P, P)),
            in1=iota_f[:], op=mybir.AluOpType.is_equal)
        nc.tensor.matmul(
            out=bank[:, :],
            lhsT=s_t[:], rhs=gt[:, slot, :],
            start=(first[w] == ci), stop=(last[w] == ci))
        if last[w] == ci:
            evict_fn(w, bank[:, 0:F], bank[:, F:2 * F])
            del acc_tiles[w]


def build_kernel(plan, n_real, debug=False):
    SH, NP, W = plan["SH"], plan["NP"], plan["W"]
    NCH, S = plan["NCH"], plan["S"]
    NT = SH // P  # node tiles per core (== W)

    nc = bacc.Bacc("TRN2", target_bir_lowering=False, name="dgi", num_devices=C)
    groups = [list(range(C))]

    # ---- I/O ----
    t_x = nc.dram_tensor("x_sh", [SH, FIN], F32, kind="ExternalInput")
    t_xp = nc.dram_tensor("xp_sh", [SH, FIN], F32, kind="ExternalInput")
    t_W1 = nc.dram_tensor("W1", [FIN, F], F32, kind="ExternalInput")
    t_W2 = nc.dram_tensor("W2", [F, F], F32, kind="ExternalInput")
    t_Wd = nc.dram_tensor("Wd0", [F, F], F32, kind="ExternalInput")
    t_b1 = nc.dram_tensor("b1", [F], F32, kind="ExternalInput")
    t_b2 = nc.dram_tensor("b2", [F], F32, kind="ExternalInput")
    t_bd = nc.dram_tensor("bd", [1], F32, kind="ExternalInput")
    t_deg = nc.dram_tensor("deg_w", [P, W], F32, kind="ExternalInput")
    t_mask = nc.dram_tensor("mask_w", [P, W], F32, kind="ExternalInput")
    t_iota = nc.dram_tensor("iota", [P, P], F32, kind="ExternalInput")
    t_ident = nc.dram_tensor("ident", [P, P], F32, kind="ExternalInput")
    t_idx = nc.dram_tensor("idx_wr", [P, S // 16], I16, kind="ExternalInput")
    t_dstv = nc.dram_tensor("dstv", [P, NCH], F32, kind="ExternalInput")
    t_pos = nc.dram_tensor("pos_sh", [SH, 1], F32, kind="ExternalOutput")
    t_neg = nc.dram_tensor("neg_sh", [SH, 1], F32, kind="ExternalOutput")
    if debug:
        t_dz13 = nc.dram_tensor("dbg_z13", [SH, 2 * F], BF16, kind="ExternalOutput")
        t_dz24 = nc.dram_tensor("dbg_z24", [SH, 2 * F], BF16, kind="ExternalOutput")
        t_dH = nc.dram_tensor("dbg_H", [SH, F], F32, kind="ExternalOutput")
        t_dHc = nc.dram_tensor("dbg_Hc", [SH, F], F32, kind="ExternalOutput")
        t_dar = nc.dram_tensor("dbg_ar", [P, 1], F32, kind="ExternalOutput")
        t_dws = nc.dram_tensor("dbg_ws", [1, F], F32, kind="ExternalOutput")

    # ---- internal DRAM ----
    z13_sh = nc.dram_tensor("z13_sh", [SH, 2 * F], BF16)
    z13_full = nc.dram_tensor("z13_full", [NP, 2 * F], BF16)
    z24_sh = nc.dram_tensor("z24_sh", [SH, 2 * F], BF16)
    z24_full = nc.dram_tensor("z24_full", [NP, 2 * F], BF16)
    H_sh = nc.dram_tensor("H_sh", [SH, F], F32)
    Hc_sh = nc.dram_tensor("Hc_sh", [SH, F], F32)
    ar_in = nc.dram_tensor("ar_in", [P, 1], F32)
    ar_out = nc.dram_tensor("ar_out", [P, 1], F32)
    ws_dram = nc.dram_tensor("ws_dram", [1, F], F32)

    with tile.TileContext(nc) as tc:
        with tc.tile_pool(name="const", bufs=1) as cp:
            nc.gpsimd.load_library(mlp_lib)
            iota_f = cp.tile([P, P], F32)
            nc.sync.dma_start(iota_f[:], t_iota[:, :])
            ident = cp.tile([P, P], F32)
            nc.sync.dma_start(ident[:], t_ident[:, :])
            b1r = cp.tile([P, F], F32)
            nc.sync.dma_start(b1r[:], t_b1.ap()[None, :].to_broadcast((P, F)))
            b2r = cp.tile([P, F], F32)
            nc.sync.dma_start(b2r[:], t_b2.ap()[None, :].to_broadcast((P, F)))
            bdr = cp.tile([P, 1], F32)
            nc.sync.dma_start(bdr[:], t_bd.ap()[None, :].to_broadcast((P, 1)))
            W2sb = cp.tile([P, F], F32)
            nc.sync.dma_start(W2sb[:], t_W2[:, :])
            W1sb = cp.tile([P, 4, F], F32)
            for k in range(4):
                nc.sync.dma_start(W1sb[:, k, :], t_W1[k * P:(k + 1) * P, :])
            deg_sb = cp.tile([P, W], F32)
            nc.sync.dma_start(deg_sb[:], t_deg[:, :])
            dinv_sb = cp.tile([P, W], F32)
            nc.vector.reciprocal(dinv_sb[:], deg_sb[:])
            nc.scalar.activation(dinv_sb[:], dinv_sb[:],
                                 mybir.ActivationFunctionType.Sqrt)
            mask_sb = cp.tile([P, W], F32)
            nc.sync.dma_start(mask_sb[:], t_mask[:, :])
            dstv_sb = cp.tile([P, NCH], F32)
            nc.sync.dma_start(dstv_sb[:], t_dstv[:, :])
            # WdT for later
            wd_sb = cp.tile([P, F], F32)
            nc.sync.dma_start(wd_sb[:], t_Wd[:, :])

            # ---------------- pass 0: z13 = [dinv*x@W1 | dinv*xp@W1] --------
            with (
                tc.tile_pool(name="p0", bufs=3) as p0,
                tc.tile_pool(name="p0ps", bufs=2, space="PSUM") as p0ps,
                tc.tile_pool(name="p0tp", bufs=2, space="PSUM") as p0tp,
            ):
                for t in range(NT):
                    for src_t, col in ((t_x, 0), (t_xp, F)):
                        xt = p0.tile([P, FIN], F32, tag="xt")
                        nc.sync.dma_start(xt[:], src_t[t * P:(t + 1) * P, :])
                        yp = p0ps.tile([P, F], F32, tag="yp")
                        for k in range(4):
                            tp = p0tp.tile([P, P], F32, tag="tp")
                            nc.tensor.transpose(out=tp[:], in_=xt[:, k * P:(k + 1) * P],
                                                identity=ident[:])
                            xT = p0.tile([P, P], F32, tag="xT")
                            nc.vector.tensor_copy(xT[:], tp[:])
                            nc.tensor.matmul(out=yp[:], lhsT=xT[:], rhs=W1sb[:, k, :],
                                             start=(k == 0), stop=(k == 3))
                        zb = p0.tile([P, F], BF16, tag="zb")
                        nc.vector.tensor_scalar_mul(zb[:], yp[:], dinv_sb[:, t:t + 1])
                        nc.sync.dma_start(z13_sh[t * P:(t + 1) * P, col:col + F], zb[:])

            nc.gpsimd.collective_compute(
                "AllGather", mybir.AluOpType.bypass, replica_groups=groups,
                ins=[z13_sh.ap().opt()], outs=[z13_full.ap().opt()])

            # ---------------- pass 1: conv1 -> z24 --------------------------
            with (
                tc.tile_pool(name="g1", bufs=3) as gath_pool,
                tc.tile_pool(name="s1", bufs=4) as s_pool,
                tc.tile_pool(name="i1", bufs=3) as idxt_pool,
                tc.tile_pool(name="e1", bufs=3) as ev_pool,
                tc.tile_pool(name="a1", bufs=4, space="PSUM") as acc_pool,
                tc.tile_pool(name="t1", bufs=2, space="PSUM") as tp_pool,
                tc.tile_pool(name="z1p", bufs=2, space="PSUM") as zp_pool,
            ):
                def evict1(w, accA, accB):
                    for acc_ap, col in ((accA, 0), (accB, F)):
                        h = ev_pool.tile([P, F], F32, tag="h")
                        nc.vector.tensor_scalar_mul(h[:], acc_ap, dinv_sb[:, w:w + 1])
                        nc.vector.tensor_add(h[:], h[:], b1r[:])
                        nc.scalar.activation(h[:], h[:],
                                             mybir.ActivationFunctionType.Relu)
                        tp = tp_pool.tile([P, P], F32, tag="tp")
                        nc.tensor.transpose(out=tp[:], in_=h[:], identity=ident[:])
                        hT = ev_pool.tile([P, P], F32, tag="hT")
                        nc.vector.tensor_copy(hT[:], tp[:])
                        zp = zp_pool.tile([P, F], F32, tag="zp")
                        nc.tensor.matmul(out=zp[:], lhsT=hT[:], rhs=W2sb[:],
                                         start=True, stop=True)
                        zb = ev_pool.tile([P, F], BF16, tag="zb")
                        nc.vector.tensor_scalar_mul(zb[:], zp[:], dinv_sb[:, w:w + 1])
                        nc.sync.dma_start(z24_sh[w * P:(w + 1) * P, col:col + F], zb[:])

                conv_pass(nc, tc, plan, z13_full, dinv_sb, iota_f, dstv_sb, t_idx,
                          (gath_pool, s_pool, idxt_pool, acc_pool), evict1)

            nc.gpsimd.collective_compute(
                "AllGather", mybir.AluOpType.bypass, replica_groups=groups,
                ins=[z24_sh.ap().opt()], outs=[z24_full.ap().opt()])

            # ---------------- pass 2: conv2 -> H, Hc, readout ---------------
            with (
                tc.tile_pool(name="g2", bufs=3) as gath_pool,
                tc.tile_pool(name="s2", bufs=4) as s_pool,
                tc.tile_pool(name="i2", bufs=3) as idxt_pool,
                tc.tile_pool(name="e2", bufs=3) as ev_pool,
                tc.tile_pool(name="a2", bufs=4, space="PSUM") as acc_pool,
                tc.tile_pool(name="r2", bufs=1, space="PSUM") as rs_pool,
            ):
                rsum = rs_pool.tile([P, 1], F32)
                seen = {"n": 0}

                def evict2(w, accA, accB):
                    Hs = ev_pool.tile([P, F], F32, tag="Hs")
                    nc.vector.tensor_scalar_mul(Hs[:], accA, dinv_sb[:, w:w + 1])
                    nc.vector.tensor_add(Hs[:], Hs[:], b2r[:])
                    nc.sync.dma_start(H_sh[w * P:(w + 1) * P, :], Hs[:])
                    nc.tensor.matmul(out=rsum[:], lhsT=Hs[:],
                                     rhs=mask_sb[:, w:w + 1],
                                     start=(seen["n"] == 0), stop=(seen["n"] == W - 1))
                    seen["n"] += 1
                    Hc = ev_pool.tile([P, F], F32, tag="Hc")
                    nc.vector.tensor_scalar_mul(Hc[:], accB, dinv_sb[:, w:w + 1])
                    nc.vector.tensor_add(Hc[:], Hc[:], b2r[:])
                    nc.sync.dma_start(Hc_sh[w * P:(w + 1) * P, :], Hc[:])

                conv_pass(nc, tc, plan, z24_full, dinv_sb, iota_f, dstv_sb, t_idx,
                          (gath_pool, s_pool, idxt_pool, acc_pool), evict2)

                rs_sb = ev_pool.tile([P, 1], F32, tag="rs")
                nc.vector.tensor_copy(rs_sb[:], rsum[:])
                nc.sync.dma_start(ar_in[:, :], rs_sb[:])

            nc.gpsimd.collective_compute(
                "AllReduce", mybir.AluOpType.add, replica_groups=groups,
                ins=[ar_in.ap().opt()], outs=[ar_out.ap().opt()])

            # ---------------- final: s, Ws, pos/neg -------------------------
            with (
                tc.tile_pool(name="fin", bufs=3) as fp,
                tc.tile_pool(name="fps", bufs=2, space="PSUM") as fps,
            ):
                s_sb = fp.tile([P, 1], F32)
                nc.sync.dma_start(s_sb[:], ar_out[:, :])
                nc.scalar.activation(s_sb[:], s_sb[:],
                                     mybir.ActivationFunctionType.Sigmoid,
                                     scale=1.0 / float(n_real))
                tpw = fps.tile([P, P], F32, tag="tpw")
                nc.tensor.transpose(out=tpw[:], in_=wd_sb[:], identity=ident[:])
                wdT = fp.tile([P, F], F32)
                nc.vector.tensor_copy(wdT[:], tpw[:])
                # Ws as a row: [1,o] = s.T @ Wd0.T
                wsp = fps.tile([1, F], F32, tag="wsp")
                nc.tensor.matmul(out=wsp[:], lhsT=s_sb[:], rhs=wdT[:],
                                 start=True, stop=True)
                ws_row = fp.tile([1, F], F32)
                nc.vector.tensor_copy(ws_row[:], wsp[:])
                nc.sync.dma_start(ws_dram[0:1, :], ws_row[:])
                ws4 = fp.tile([P, 4, F], F32)
                for k in range(4):
                    nc.sync.dma_start(ws4[:, k, :],
                                      ws_dram.ap()[0:1, :].to_broadcast((P, F)))
                # pos/neg windows, 4 at a time
                for (h_dram, o_dram) in ((H_sh, t_pos), (Hc_sh, t_neg)):
                    for q in range(0, W, 4):
                        nw = min(4, W - q)
                        ht = fp.tile([P, 4, F], F32, tag="ht")
                        nc.sync.dma_start(
                            ht[:, :nw, :],
                            h_dram.ap()[q * P:(q + nw) * P, :]
                            .rearrange("(k p) f -> p k f", p=P))
                        pr = fp.tile([P, 4, F], F32, tag="pr")
                        nc.vector.tensor_mul(pr[:, :nw, :], ht[:, :nw, :],
                                             ws4[:, :nw, :])
                        po = fp.tile([P, 4], F32, tag="po")
                        nc.vector.reduce_sum(po[:, :nw], pr[:, :nw, :],
                                             axis=mybir.AxisListType.X)
                        nc.vector.tensor_scalar_add(po[:, :nw], po[:, :nw],
                                                    bdr[:, 0:1])
                        nc.sync.dma_start(
                            o_dram.ap()[q * P:(q + nw) * P, :]
                            .rearrange("(k p) f -> p k f", p=P)[:, :, 0],
                            po[:, :nw])

                if debug:
                    nc.sync.dma_start(t_dz13.ap(), z13_sh.ap())
                    nc.sync.dma_start(t_dz24.ap(), z24_sh.ap())
                    nc.sync.dma_start(t_dH.ap(), H_sh.ap())
                    nc.sync.dma_start(t_dHc.ap(), Hc_sh.ap())
                    nc.sync.dma_start(t_dar.ap(), ar_out.ap())
                    nc.sync.dma_start(t_dws.ap(), ws_dram.ap())

    nc.compile()
    return nc


# ------------------------------------------------------------------- driver --
def run(x, edge_index, perm, W1, b1, W2, b2, Wd, bd, n_real, w_per_core,
        bucket_rows, debug=False):
    SH = w_per_core * P
    NP = SH * C
    src = edge_index[0].astype(np.int64)
    dst = edge_index[1].astype(np.int64)
    loops = np.arange(n_real, dtype=np.int64)
    src = np.concatenate([src, loops])
    dst = np.concatenate([dst, loops])

    plan = build_plan(src, dst, n_real, w_per_core, bucket_rows)

    deg = np.bincount(dst, minlength=NP).astype(np.float32)
    deg[n_real:] = 1.0
    mask = np.zeros(NP, np.float32)
    mask[:n_real] = 1.0
    xpad = np.zeros((NP, FIN), np.float32)
    xpad[:n_real] = x
    xppad = np.zeros((NP, FIN), np.float32)
    xppad[:n_real] = x[perm]

    iota = np.tile(np.arange(P, dtype=np.float32), (P, 1))
    ident = np.eye(P, dtype=np.float32)

    in_maps = []
    for c in range(C):
        sl = slice(c * SH, (c + 1) * SH)
        in_maps.append({
            "x_sh": xpad[sl], "xp_sh": xppad[sl],
            "W1": W1, "W2": W2, "Wd0": Wd[0], "b1": b1, "b2": b2,
            "bd": bd.astype(np.float32),
            "deg_w": deg[sl].reshape(w_per_core, P).T.copy(),
            "mask_w": mask[sl].reshape(w_per_core, P).T.copy(),
            "iota": iota, "ident": ident,
            "idx_wr": plan["idx_wr"][c], "dstv": plan["dstv"][c],
        })

    nc = build_kernel(plan, n_real, debug=debug)
    res = run_bass_kernel_spmd(nc, in_maps, core_ids=list(range(C)))
    pos = np.concatenate([res.results[c]["pos_sh"] for c in range(C)])[:n_real]
    neg = np.concatenate([res.results[c]["neg_sh"] for c in range(C)])[:n_real]
    if debug:
        return pos, neg, res
    return pos, neg


# ----------------------------------------------------------------- entrypoint --
def kernel(x, edge_index, perm, W1, b1, W2, b2, Wd, bd):
    """DGI forward on 8 trn2 cores. Returns (pos, neg) like the reference."""
    pos, neg = run(np.asarray(x, np.float32), np.asarray(edge_index),
                   np.asarray(perm), np.asarray(W1, np.float32),
                   np.asarray(b1, np.float32), np.asarray(W2, np.float32),
                   np.asarray(b2, np.float32), np.asarray(Wd, np.float32),
                   np.asarray(bd, np.float32),
                   n_real=100000, w_per_core=98, bucket_rows=32768)
    return pos, neg

